# revision 1
# baseline (speedup 1.0000x reference)
"""ACT halting-weights kernel for 8 TRN2 NeuronCores (pure data parallel over B)."""

import sys

for _p in ("/opt/trn_rl_repo", "/root/.axon_site"):
    if _p not in sys.path:
        sys.path.insert(0, _p)

import numpy as np

B, T, D = 256, 64, 2048
NCORES = 8
BL = B // NCORES          # 32 rows per core
P = 128                   # SBUF partitions
PAIRS = BL // 2           # 16 row-pairs per core; each pair = 128 partitions of (b, t)
NCHUNK = 512              # fp32 PSUM bank width
THRESHOLD = 0.99
EPSILON = 0.01

_CACHE = {}


def _build():
    import concourse.tile as tile
    from concourse import bacc, mybir

    f32 = mybir.dt.float32
    Alu = mybir.AluOpType

    nc = bacc.Bacc()
    hp_d = nc.dram_tensor("halt_probs", [BL, T, 1], f32, kind="ExternalInput")
    out_d = nc.dram_tensor("outputs", [BL, T, D], f32, kind="ExternalInput")
    sw_d = nc.dram_tensor("step_weights", [BL, T], f32, kind="ExternalInput")
    fin_d = nc.dram_tensor("final", [BL, D], f32, kind="ExternalOutput")
    pond_d = nc.dram_tensor("ponder", [BL, 1], f32, kind="ExternalOutput")
    w_d = nc.dram_tensor("weights", [BL, T], f32, kind="ExternalOutput")
    wtmp_d = nc.dram_tensor("wtmp", [BL * T], f32, kind="Internal")

    # Block-diagonal placement masks: for pair m, column 2m+g (within that
    # pair's [P, BL] lhsT slice) is 1 on partitions [64g, 64g+64).
    masks_np = np.zeros((P, PAIRS * BL), np.float32)
    for m in range(PAIRS):
        masks_np[0:64, m * BL + 2 * m] = 1.0
        masks_np[64:128, m * BL + 2 * m + 1] = 1.0
    masks_d = nc.inline_tensor(masks_np, name="masks")
    steps_np = np.broadcast_to(
        np.arange(1, T + 1, dtype=np.float32), (BL, T)
    ).copy()
    steps_d = nc.inline_tensor(steps_np, name="steps")

    with tile.TileContext(nc) as tc:
        with (
            tc.tile_pool(name="small", bufs=1) as small,
            tc.tile_pool(name="rhs", bufs=6) as rhsp,
            tc.tile_pool(name="psum", bufs=1, space="PSUM") as psump,
            tc.tile_pool(name="fout", bufs=1) as foutp,
        ):
            # ---- Phase A: per-row halting weights ([BL, T], b on partitions)
            # Small DMAs ride the ACT HWDGE ring so they never queue behind
            # the 1 MB outputs stream on the SP ring.
            hp = small.tile([BL, T], f32)
            nc.scalar.dma_start(hp[:], hp_d[:].rearrange("b t one -> b (t one)"))
            sw = small.tile([BL, T], f32)
            nc.scalar.dma_start(sw[:], sw_d[:])
            steps_sb = small.tile([BL, T], f32)
            nc.scalar.dma_start(steps_sb[:], steps_d[:])
            masks_sb = small.tile([P, PAIRS * BL], f32)
            nc.scalar.dma_start(masks_sb[:], masks_d[:])

            cum = small.tile([BL, T], f32)
            nc.vector.tensor_tensor_scan(
                cum[:], hp[:], hp[:], 0.0, Alu.add, Alu.bypass
            )
            # E = (cum >= THRESHOLD), then force last step -> halting mask E'
            E = small.tile([BL, T], f32)
            nc.vector.tensor_scalar(
                out=E[:], in0=cum[:], scalar1=THRESHOLD, scalar2=None, op0=Alu.is_ge
            )
            nc.vector.memset(E[:, T - 1 : T], 1.0)
            # at[t] = E'[t] - E'[t-1]  (first crossing; E' is monotone)
            at = small.tile([BL, T], f32)
            nc.vector.tensor_copy(at[:, 0:1], E[:, 0:1])
            nc.vector.tensor_sub(at[:, 1:T], E[:, 1:T], E[:, 0 : T - 1])
            # remaining(t) = 1 - cum[t] + hp[t]
            rem = small.tile([BL, T], f32)
            nc.vector.tensor_sub(rem[:], hp[:], cum[:])
            nc.vector.tensor_scalar_add(rem[:], rem[:], 1.0)
            # w_pre = hp * (1 - E') + rem * at
            notE = small.tile([BL, T], f32)
            nc.vector.tensor_scalar(
                out=notE[:], in0=E[:], scalar1=-1.0, scalar2=1.0,
                op0=Alu.mult, op1=Alu.add,
            )
            w1 = small.tile([BL, T], f32)
            nc.vector.tensor_mul(w1[:], hp[:], notE[:])
            w2 = small.tile([BL, T], f32)
            nc.vector.tensor_mul(w2[:], rem[:], at[:])
            wp = small.tile([BL, T], f32)
            nc.vector.tensor_add(wp[:], w1[:], w2[:])
            nc.vector.tensor_mul(wp[:], wp[:], sw[:])
            # normalize: w / max(sum(w), EPS)
            sums = small.tile([BL, 1], f32)
            nc.vector.reduce_sum(sums[:], wp[:], axis=mybir.AxisListType.X)
            nc.vector.tensor_scalar_max(sums[:], sums[:], EPSILON)
            inv = small.tile([BL, 1], f32)
            nc.vector.reciprocal(inv[:], sums[:])
            wgt = small.tile([BL, T], f32)
            nc.vector.tensor_scalar_mul(wgt[:], wp[:], inv[:])

            nc.scalar.dma_start(w_d[:], wgt[:])
            nc.scalar.dma_start(wtmp_d[:].rearrange("(b t) -> b t", t=T), wgt[:])
            # ponder = sum(weights * (1..T))
            pond_t = small.tile([BL, T], f32)
            nc.vector.tensor_mul(pond_t[:], wgt[:], steps_sb[:])
            pond = small.tile([BL, 1], f32)
            nc.vector.reduce_sum(pond[:], pond_t[:], axis=mybir.AxisListType.X)
            nc.scalar.dma_start(pond_d[:], pond[:])

            # wstack[64g + t, m] = weights[2m + g, t]: read the row-major
            # weights back with t-major partitions (transpose via DRAM).
            wstack = small.tile([P, PAIRS], f32)
            nc.scalar.dma_start(
                wstack[:], wtmp_d[:].rearrange("(m p) -> p m", p=P)
            )
            # Block-diagonal lhsT for all pairs: bd_all = masks * wstack[:, m]
            bd_all = small.tile([P, PAIRS * BL], f32)
            nc.vector.tensor_tensor(
                bd_all[:].rearrange("p (m c) -> p m c", c=BL),
                masks_sb[:].rearrange("p (m c) -> p m c", c=BL),
                wstack[:].unsqueeze(2).broadcast_to([P, PAIRS, BL]),
                Alu.mult,
            )

            # ---- Phase B: final[b, d] = sum_t wgt[b, t] * outputs[b, t, d]
            outs_flat = out_d[:].rearrange("b t d -> (b t) d")  # [BL*T, D]
            psum_fin = psump.tile([BL, D], f32)
            for m in range(PAIRS):
                rhs = rhsp.tile([P, D], f32)
                nc.sync.dma_start(rhs[:], outs_flat[m * P : (m + 1) * P, :])
                for j in range(D // NCHUNK):
                    nc.tensor.matmul(
                        psum_fin[:, j * NCHUNK : (j + 1) * NCHUNK],
                        bd_all[:, m * BL : (m + 1) * BL],
                        rhs[:, j * NCHUNK : (j + 1) * NCHUNK],
                        start=(m == 0),
                        stop=(m == PAIRS - 1),
                    )
            fin_sb = foutp.tile([BL, D], f32)
            nc.vector.tensor_copy(fin_sb[:], psum_fin[:])
            nc.sync.dma_start(fin_d[:], fin_sb[:])

    nc.finalize()
    return nc


def kernel(halt_probs, outputs, step_weights):
    from concourse.bass_utils import run_bass_kernel_spmd

    halt_probs = np.ascontiguousarray(np.asarray(halt_probs, dtype=np.float32))
    outputs = np.ascontiguousarray(np.asarray(outputs, dtype=np.float32))
    step_weights = np.ascontiguousarray(np.asarray(step_weights, dtype=np.float32))

    if "nc" not in _CACHE:
        _CACHE["nc"] = _build()
    nc = _CACHE["nc"]

    core_ids = list(range(NCORES))
    in_maps = []
    for i in core_ids:
        s = slice(i * BL, (i + 1) * BL)
        in_maps.append(
            {
                "halt_probs": np.ascontiguousarray(halt_probs[s]),
                "outputs": np.ascontiguousarray(outputs[s]),
                "step_weights": np.ascontiguousarray(step_weights[s]),
            }
        )
    res = run_bass_kernel_spmd(nc, in_maps, core_ids)
    final = np.concatenate([res.results[i]["final"] for i in core_ids], axis=0)
    ponder = np.concatenate(
        [res.results[i]["ponder"][:, 0] for i in core_ids], axis=0
    )
    weights = np.concatenate([res.results[i]["weights"] for i in core_ids], axis=0)
    return final, ponder, weights


# revision 5
# speedup vs baseline: 1.3641x; 1.3641x over previous
"""ACT halting-weights kernel for 8 TRN2 NeuronCores (pure data parallel over B)."""

import sys

for _p in ("/opt/trn_rl_repo", "/root/.axon_site"):
    if _p not in sys.path:
        sys.path.insert(0, _p)

import numpy as np

B, T, D = 256, 64, 2048
NCORES = 8
BL = B // NCORES          # 32 rows per core
P = 128                   # SBUF partitions
PAIRS = BL // 2           # 16 row-pairs per core; each pair = 128 partitions of (b, t)
NCHUNK = 512              # fp32 PSUM bank width
THRESHOLD = 0.99
EPSILON = 0.01

_CACHE = {}


def _build():
    import concourse.tile as tile
    from concourse import bacc, mybir

    f32 = mybir.dt.float32
    Alu = mybir.AluOpType

    nc = bacc.Bacc()
    hp_d = nc.dram_tensor("halt_probs", [BL, T, 1], f32, kind="ExternalInput")
    out_d = nc.dram_tensor("outputs", [BL, T, D], f32, kind="ExternalInput")
    sw_d = nc.dram_tensor("step_weights", [BL, T], f32, kind="ExternalInput")
    fin_d = nc.dram_tensor("final", [BL, D], f32, kind="ExternalOutput")
    pond_d = nc.dram_tensor("ponder", [BL, 1], f32, kind="ExternalOutput")
    w_d = nc.dram_tensor("weights", [BL, T], f32, kind="ExternalOutput")
    wtmp_d = nc.dram_tensor("wtmp", [BL * T], f32, kind="Internal")

    # Block-diagonal placement masks: for pair m, column 2m+g (within that
    # pair's [P, BL] lhsT slice) is 1 on partitions [64g, 64g+64).
    masks_np = np.zeros((P, PAIRS * BL), np.float32)
    for m in range(PAIRS):
        masks_np[0:64, m * BL + 2 * m] = 1.0
        masks_np[64:128, m * BL + 2 * m + 1] = 1.0
    masks_d = nc.inline_tensor(masks_np, name="masks")
    steps_np = np.broadcast_to(
        np.arange(1, T + 1, dtype=np.float32), (BL, T)
    ).copy()
    steps_d = nc.inline_tensor(steps_np, name="steps")

    with tile.TileContext(nc) as tc:
        with (
            tc.tile_pool(name="small", bufs=1) as small,
            tc.tile_pool(name="rhs", bufs=6) as rhsp,
            tc.tile_pool(name="psum", bufs=1, space="PSUM") as psump,
            tc.tile_pool(name="fout", bufs=1) as foutp,
        ):
            # ---- Phase A: per-row halting weights ([BL, T], b on partitions)
            # Small DMAs ride the ACT HWDGE ring so they never queue behind
            # the 1 MB outputs stream on the SP ring.
            hp = small.tile([BL, T], f32)
            nc.scalar.dma_start(hp[:], hp_d[:].rearrange("b t one -> b (t one)"))
            sw = small.tile([BL, T], f32)
            nc.scalar.dma_start(sw[:], sw_d[:])
            steps_sb = small.tile([BL, T], f32)
            nc.scalar.dma_start(steps_sb[:], steps_d[:])
            masks_sb = small.tile([P, PAIRS * BL], f32)
            nc.scalar.dma_start(masks_sb[:], masks_d[:])

            cum = small.tile([BL, T], f32)
            nc.vector.tensor_tensor_scan(
                cum[:], hp[:], hp[:], 0.0, Alu.add, Alu.bypass
            )
            # E = (cum >= THRESHOLD), then force last step -> halting mask E'
            E = small.tile([BL, T], f32)
            nc.vector.tensor_scalar(
                out=E[:], in0=cum[:], scalar1=THRESHOLD, scalar2=None, op0=Alu.is_ge
            )
            nc.vector.memset(E[:, T - 1 : T], 1.0)
            # at[t] = E'[t] - E'[t-1]  (first crossing; E' is monotone)
            at = small.tile([BL, T], f32)
            nc.vector.tensor_copy(at[:, 0:1], E[:, 0:1])
            nc.vector.tensor_sub(at[:, 1:T], E[:, 1:T], E[:, 0 : T - 1])
            # remaining(t) = 1 - cum[t] + hp[t]
            rem = small.tile([BL, T], f32)
            nc.vector.tensor_sub(rem[:], hp[:], cum[:])
            nc.vector.tensor_scalar_add(rem[:], rem[:], 1.0)
            # w_pre = hp * (1 - E') + rem * at
            notE = small.tile([BL, T], f32)
            nc.vector.tensor_scalar(
                out=notE[:], in0=E[:], scalar1=-1.0, scalar2=1.0,
                op0=Alu.mult, op1=Alu.add,
            )
            w1 = small.tile([BL, T], f32)
            nc.vector.tensor_mul(w1[:], hp[:], notE[:])
            w2 = small.tile([BL, T], f32)
            nc.vector.tensor_mul(w2[:], rem[:], at[:])
            wp = small.tile([BL, T], f32)
            nc.vector.tensor_add(wp[:], w1[:], w2[:])
            nc.vector.tensor_mul(wp[:], wp[:], sw[:])
            # normalize: w / max(sum(w), EPS)
            sums = small.tile([BL, 1], f32)
            nc.vector.reduce_sum(sums[:], wp[:], axis=mybir.AxisListType.X)
            nc.vector.tensor_scalar_max(sums[:], sums[:], EPSILON)
            inv = small.tile([BL, 1], f32)
            nc.vector.reciprocal(inv[:], sums[:])
            wgt = small.tile([BL, T], f32)
            nc.vector.tensor_scalar_mul(wgt[:], wp[:], inv[:])

            nc.scalar.dma_start(wtmp_d[:].rearrange("(b t) -> b t", t=T), wgt[:])
            # wstack[64g + t, m] = weights[2m + g, t]: read the row-major
            # weights back with t-major partitions (transpose via DRAM).
            wstack = small.tile([P, PAIRS], f32)
            nc.scalar.dma_start(
                wstack[:], wtmp_d[:].rearrange("(m p) -> p m", p=P)
            )
            # Non-critical small outputs after the latency-critical roundtrip.
            nc.scalar.dma_start(w_d[:], wgt[:])
            # ponder = sum(weights * (1..T))
            pond_t = small.tile([BL, T], f32)
            nc.vector.tensor_mul(pond_t[:], wgt[:], steps_sb[:])
            pond = small.tile([BL, 1], f32)
            nc.vector.reduce_sum(pond[:], pond_t[:], axis=mybir.AxisListType.X)
            nc.scalar.dma_start(pond_d[:], pond[:])
            # Block-diagonal lhsT for all pairs: bd_all = masks * wstack[:, m]
            # (written as float32r so the PE can stream it single-pass)
            f32r = mybir.dt.float32r
            bd_all = small.tile([P, PAIRS * BL], f32r)
            nc.vector.tensor_tensor(
                bd_all[:].rearrange("p (m c) -> p m c", c=BL),
                masks_sb[:].rearrange("p (m c) -> p m c", c=BL),
                wstack[:].unsqueeze(2).broadcast_to([P, PAIRS, BL]),
                Alu.mult,
            )

            # ---- Phase B: final[b, d] = sum_t wgt[b, t] * outputs[b, t, d]
            outs_flat = out_d[:].rearrange("b t d -> (b t) d")  # [BL*T, D]
            psum_fin = psump.tile([BL, D], f32)
            for m in range(PAIRS):
                rhs = rhsp.tile([P, D], f32r)
                nc.sync.dma_start(
                    rhs[:], outs_flat[m * P : (m + 1) * P, :].bitcast(f32r)
                )
                for j in range(D // NCHUNK):
                    # float32r: fp32 data, single-pass PE streaming (4x fp32)
                    nc.tensor.matmul(
                        psum_fin[:, j * NCHUNK : (j + 1) * NCHUNK],
                        bd_all[:, m * BL : (m + 1) * BL],
                        rhs[:, j * NCHUNK : (j + 1) * NCHUNK],
                        start=(m == 0),
                        stop=(m == PAIRS - 1),
                    )
            fin_sb = foutp.tile([BL, D], f32)
            nc.vector.tensor_copy(fin_sb[:], psum_fin[:])
            nc.sync.dma_start(fin_d[:], fin_sb[:])

    nc.finalize()
    return nc


def kernel(halt_probs, outputs, step_weights):
    from concourse.bass_utils import run_bass_kernel_spmd

    halt_probs = np.ascontiguousarray(np.asarray(halt_probs, dtype=np.float32))
    outputs = np.ascontiguousarray(np.asarray(outputs, dtype=np.float32))
    step_weights = np.ascontiguousarray(np.asarray(step_weights, dtype=np.float32))

    if "nc" not in _CACHE:
        _CACHE["nc"] = _build()
    nc = _CACHE["nc"]

    core_ids = list(range(NCORES))
    in_maps = []
    for i in core_ids:
        s = slice(i * BL, (i + 1) * BL)
        in_maps.append(
            {
                "halt_probs": np.ascontiguousarray(halt_probs[s]),
                "outputs": np.ascontiguousarray(outputs[s]),
                "step_weights": np.ascontiguousarray(step_weights[s]),
            }
        )
    res = run_bass_kernel_spmd(nc, in_maps, core_ids)
    final = np.concatenate([res.results[i]["final"] for i in core_ids], axis=0)
    ponder = np.concatenate(
        [res.results[i]["ponder"][:, 0] for i in core_ids], axis=0
    )
    weights = np.concatenate([res.results[i]["weights"] for i in core_ids], axis=0)
    return final, ponder, weights


# revision 11
# speedup vs baseline: 1.4356x; 1.0524x over previous
"""ACT halting-weights kernel for 8 TRN2 NeuronCores (pure data parallel over B)."""

import sys

for _p in ("/opt/trn_rl_repo", "/root/.axon_site"):
    if _p not in sys.path:
        sys.path.insert(0, _p)

import numpy as np

B, T, D = 256, 64, 2048
NCORES = 8
BL = B // NCORES          # 32 rows per core
P = 128                   # SBUF partitions
PAIRS = BL // 2           # 16 row-pairs per core; each pair = 128 partitions of (b, t)
NCHUNK = 512              # fp32 PSUM bank width
THRESHOLD = 0.99
EPSILON = 0.01

_CACHE = {}


def _build():
    import concourse.tile as tile
    from concourse import bacc, mybir

    f32 = mybir.dt.float32
    Alu = mybir.AluOpType

    nc = bacc.Bacc()
    hp_d = nc.dram_tensor("halt_probs", [BL, T, 1], f32, kind="ExternalInput")
    out_d = nc.dram_tensor("outputs", [BL, T, D], f32, kind="ExternalInput")
    sw_d = nc.dram_tensor("step_weights", [BL, T], f32, kind="ExternalInput")
    fin_d = nc.dram_tensor("final", [BL, D], f32, kind="ExternalOutput")
    pond_d = nc.dram_tensor("ponder", [BL, 1], f32, kind="ExternalOutput")
    w_d = nc.dram_tensor("weights", [BL, T], f32, kind="ExternalOutput")

    # Block-diagonal placement masks: for pair m, column 2m+g (within that
    # pair's [P, BL] lhsT slice) is 1 on partitions [64g, 64g+64).
    masks_np = np.zeros((P, PAIRS * BL), np.float32)
    for m in range(PAIRS):
        masks_np[0:64, m * BL + 2 * m] = 1.0
        masks_np[64:128, m * BL + 2 * m + 1] = 1.0
    masks_d = nc.inline_tensor(masks_np, name="masks")
    steps_np = np.broadcast_to(
        np.arange(1, T + 1, dtype=np.float32), (BL, T)
    ).copy()
    steps_d = nc.inline_tensor(steps_np, name="steps")
    ident_d = nc.inline_tensor(np.eye(BL, dtype=np.float32), name="ident")

    with tile.TileContext(nc) as tc:
        with (
            tc.tile_pool(name="small", bufs=1) as small,
            tc.tile_pool(name="rhs", bufs=10) as rhsp,
            tc.tile_pool(name="psum", bufs=1, space="PSUM") as psump,
            tc.tile_pool(name="fout", bufs=1) as foutp,
        ):
            # ---- Phase A: per-row halting weights ([BL, T], b on partitions)
            # Small DMAs ride the ACT HWDGE ring so they never queue behind
            # the 1 MB outputs stream on the SP ring.
            hp = small.tile([BL, T], f32)
            nc.scalar.dma_start(hp[:], hp_d[:].rearrange("b t one -> b (t one)"))
            sw = small.tile([BL, T], f32)
            nc.scalar.dma_start(sw[:], sw_d[:])
            steps_sb = small.tile([BL, T], f32)
            nc.scalar.dma_start(steps_sb[:], steps_d[:])
            masks_sb = small.tile([P, PAIRS * BL], f32)
            nc.scalar.dma_start(masks_sb[:], masks_d[:])

            ident = small.tile([BL, BL], f32)
            nc.scalar.dma_start(ident[:], ident_d[:])

            cum = small.tile([BL, T], f32)
            nc.vector.tensor_tensor_scan(
                cum[:], hp[:], hp[:], 0.0, Alu.add, Alu.bypass
            )
            # E' = (cum >= THRESHOLD) with forced last step (halting mask)
            E = small.tile([BL, T], f32)
            nc.vector.tensor_scalar(
                out=E[:], in0=cum[:], scalar1=THRESHOLD, scalar2=None, op0=Alu.is_ge
            )
            nc.vector.memset(E[:, T - 1 : T], 1.0)
            # cumprev = cum - hp (= cumsum up to t-1); rem = 1 - cumprev
            cumprev = small.tile([BL, T], f32)
            nc.vector.tensor_sub(cumprev[:], cum[:], hp[:])
            # at = E' * (1 - Eprev): first step where E' holds, elementwise
            notEp = small.tile([BL, T], f32)
            nc.vector.tensor_scalar(
                out=notEp[:], in0=cumprev[:], scalar1=THRESHOLD, scalar2=None,
                op0=Alu.is_lt,
            )
            at = small.tile([BL, T], f32)
            nc.vector.tensor_mul(at[:], E[:], notEp[:])
            rem = small.tile([BL, T], f32)
            nc.vector.tensor_scalar(
                out=rem[:], in0=cumprev[:], scalar1=-1.0, scalar2=1.0,
                op0=Alu.mult, op1=Alu.add,
            )
            # w_pre = hp * (1 - E') + rem * at
            notE = small.tile([BL, T], f32)
            nc.vector.tensor_scalar(
                out=notE[:], in0=E[:], scalar1=-1.0, scalar2=1.0,
                op0=Alu.mult, op1=Alu.add,
            )
            w1 = small.tile([BL, T], f32)
            nc.vector.tensor_mul(w1[:], hp[:], notE[:])
            w2 = small.tile([BL, T], f32)
            nc.vector.tensor_mul(w2[:], rem[:], at[:])
            wp = small.tile([BL, T], f32)
            nc.vector.tensor_add(wp[:], w1[:], w2[:])
            nc.vector.tensor_mul(wp[:], wp[:], sw[:])
            # normalize: w / max(sum(w), EPS)
            sums = small.tile([BL, 1], f32)
            nc.vector.reduce_sum(sums[:], wp[:], axis=mybir.AxisListType.X)
            nc.vector.tensor_scalar_max(sums[:], sums[:], EPSILON)
            inv = small.tile([BL, 1], f32)
            nc.vector.reciprocal(inv[:], sums[:])
            wgt = small.tile([BL, T], f32)
            nc.vector.tensor_scalar_mul(wgt[:], wp[:], inv[:])

            # wstack[64g + t, m] = weights[2m + g, t], via PE transpose +
            # two DVE copies (one partition-shifted) -- no DRAM roundtrip.
            wgtT_ps = psump.tile([T, BL], f32)
            nc.tensor.transpose(wgtT_ps[:], wgt[:], ident[:])
            wgtT3 = wgtT_ps[:].rearrange("t (m g) -> t g m", g=2)
            wstack = small.tile([P, PAIRS], f32)
            nc.vector.tensor_copy(wstack[0:T, :], wgtT3[:, 0, :])
            nc.vector.tensor_copy(wstack[T:P, :], wgtT3[:, 1, :])
            # Non-critical small outputs.
            nc.scalar.dma_start(w_d[:], wgt[:])
            # ponder = sum(weights * (1..T))
            pond_t = small.tile([BL, T], f32)
            nc.vector.tensor_mul(pond_t[:], wgt[:], steps_sb[:])
            pond = small.tile([BL, 1], f32)
            nc.vector.reduce_sum(pond[:], pond_t[:], axis=mybir.AxisListType.X)
            nc.scalar.dma_start(pond_d[:], pond[:])
            # Block-diagonal lhsT for all pairs: bd_all = masks * wstack[:, m]
            # (written as float32r so the PE can stream it single-pass)
            f32r = mybir.dt.float32r
            bd_all = small.tile([P, PAIRS * BL], f32r)
            nc.vector.tensor_tensor(
                bd_all[:].rearrange("p (m c) -> p m c", c=BL),
                masks_sb[:].rearrange("p (m c) -> p m c", c=BL),
                wstack[:].unsqueeze(2).broadcast_to([P, PAIRS, BL]),
                Alu.mult,
            )

            # ---- Phase B: final[b, d] = sum_t wgt[b, t] * outputs[b, t, d]
            outs_flat = out_d[:].rearrange("b t d -> (b t) d")  # [BL*T, D]
            psum_fin = psump.tile([BL, D], f32)
            for m in range(PAIRS):
                rhs = rhsp.tile([P, D], f32r)
                nc.sync.dma_start(
                    rhs[:], outs_flat[m * P : (m + 1) * P, :].bitcast(f32r)
                )
                for j in range(D // NCHUNK):
                    # float32r: fp32 data, single-pass PE streaming (4x fp32)
                    nc.tensor.matmul(
                        psum_fin[:, j * NCHUNK : (j + 1) * NCHUNK],
                        bd_all[:, m * BL : (m + 1) * BL],
                        rhs[:, j * NCHUNK : (j + 1) * NCHUNK],
                        start=(m == 0),
                        stop=(m == PAIRS - 1),
                    )
            fin_sb = foutp.tile([BL, D], f32)
            nc.vector.tensor_copy(fin_sb[:], psum_fin[:])
            nc.sync.dma_start(fin_d[:], fin_sb[:])

    nc.finalize()
    return nc


def kernel(halt_probs, outputs, step_weights):
    from concourse.bass_utils import run_bass_kernel_spmd

    halt_probs = np.ascontiguousarray(np.asarray(halt_probs, dtype=np.float32))
    outputs = np.ascontiguousarray(np.asarray(outputs, dtype=np.float32))
    step_weights = np.ascontiguousarray(np.asarray(step_weights, dtype=np.float32))

    if "nc" not in _CACHE:
        _CACHE["nc"] = _build()
    nc = _CACHE["nc"]

    core_ids = list(range(NCORES))
    in_maps = []
    for i in core_ids:
        s = slice(i * BL, (i + 1) * BL)
        in_maps.append(
            {
                "halt_probs": np.ascontiguousarray(halt_probs[s]),
                "outputs": np.ascontiguousarray(outputs[s]),
                "step_weights": np.ascontiguousarray(step_weights[s]),
            }
        )
    res = run_bass_kernel_spmd(nc, in_maps, core_ids)
    final = np.concatenate([res.results[i]["final"] for i in core_ids], axis=0)
    ponder = np.concatenate(
        [res.results[i]["ponder"][:, 0] for i in core_ids], axis=0
    )
    weights = np.concatenate([res.results[i]["weights"] for i in core_ids], axis=0)
    return final, ponder, weights


# revision 13
# speedup vs baseline: 1.4641x; 1.0198x over previous
"""ACT halting-weights kernel for 8 TRN2 NeuronCores (pure data parallel over B)."""

import sys

for _p in ("/opt/trn_rl_repo", "/root/.axon_site"):
    if _p not in sys.path:
        sys.path.insert(0, _p)

import numpy as np

B, T, D = 256, 64, 2048
NCORES = 8
BL = B // NCORES          # 32 rows per core
P = 128                   # SBUF partitions
PAIRS = BL // 2           # 16 row-pairs per core; each pair = 128 partitions of (b, t)
NCHUNK = 512              # fp32 PSUM bank width
THRESHOLD = 0.99
EPSILON = 0.01

_CACHE = {}


def _build():
    import concourse.tile as tile
    from concourse import bacc, mybir

    f32 = mybir.dt.float32
    Alu = mybir.AluOpType

    nc = bacc.Bacc()
    hp_d = nc.dram_tensor("halt_probs", [BL, T, 1], f32, kind="ExternalInput")
    out_d = nc.dram_tensor("outputs", [BL, T, D], f32, kind="ExternalInput")
    sw_d = nc.dram_tensor("step_weights", [BL, T], f32, kind="ExternalInput")
    fin_d = nc.dram_tensor("final", [BL, D], f32, kind="ExternalOutput")
    pond_d = nc.dram_tensor("ponder", [BL, 1], f32, kind="ExternalOutput")
    w_d = nc.dram_tensor("weights", [BL, T], f32, kind="ExternalOutput")

    # Block-diagonal placement masks: for pair m, column 2m+g (within that
    # pair's [P, BL] lhsT slice) is 1 on partitions [64g, 64g+64).
    masks_np = np.zeros((P, PAIRS * BL), np.float32)
    for m in range(PAIRS):
        masks_np[0:64, m * BL + 2 * m] = 1.0
        masks_np[64:128, m * BL + 2 * m + 1] = 1.0
    masks_d = nc.inline_tensor(masks_np, name="masks")
    steps_np = np.broadcast_to(
        np.arange(1, T + 1, dtype=np.float32), (BL, T)
    ).copy()
    steps_d = nc.inline_tensor(steps_np, name="steps")
    ident_d = nc.inline_tensor(np.eye(BL, dtype=np.float32), name="ident")

    with tile.TileContext(nc) as tc:
        with (
            tc.tile_pool(name="small", bufs=1) as small,
            tc.tile_pool(name="rhs", bufs=10) as rhsp,
            tc.tile_pool(name="psum", bufs=1, space="PSUM") as psump,
            tc.tile_pool(name="fout", bufs=1) as foutp,
        ):
            # ---- Phase A: per-row halting weights ([BL, T], b on partitions)
            # Small DMAs ride the ACT HWDGE ring so they never queue behind
            # the 1 MB outputs stream on the SP ring.
            hp = small.tile([BL, T], f32)
            nc.scalar.dma_start(hp[:], hp_d[:].rearrange("b t one -> b (t one)"))
            sw = small.tile([BL, T], f32)
            nc.scalar.dma_start(sw[:], sw_d[:])
            steps_sb = small.tile([BL, T], f32)
            nc.scalar.dma_start(steps_sb[:], steps_d[:])
            masks_sb = small.tile([P, PAIRS * BL], f32)
            nc.scalar.dma_start(masks_sb[:], masks_d[:])

            ident = small.tile([BL, BL], f32)
            nc.scalar.dma_start(ident[:], ident_d[:])

            cum = small.tile([BL, T], f32)
            nc.vector.tensor_tensor_scan(
                cum[:], hp[:], hp[:], 0.0, Alu.add, Alu.bypass
            )
            # E' = (cum >= THRESHOLD) with forced last step (halting mask)
            E = small.tile([BL, T], f32)
            nc.vector.tensor_scalar(
                out=E[:], in0=cum[:], scalar1=THRESHOLD, scalar2=None, op0=Alu.is_ge
            )
            nc.vector.memset(E[:, T - 1 : T], 1.0)
            # cumprev = cum - hp (= cumsum up to t-1); rem = 1 - cumprev
            cumprev = small.tile([BL, T], f32)
            nc.vector.tensor_sub(cumprev[:], cum[:], hp[:])
            # at = E' * (1 - Eprev): first step where E' holds, elementwise
            notEp = small.tile([BL, T], f32)
            nc.vector.tensor_scalar(
                out=notEp[:], in0=cumprev[:], scalar1=THRESHOLD, scalar2=None,
                op0=Alu.is_lt,
            )
            at = small.tile([BL, T], f32)
            nc.vector.tensor_mul(at[:], E[:], notEp[:])
            rem = small.tile([BL, T], f32)
            nc.vector.tensor_scalar(
                out=rem[:], in0=cumprev[:], scalar1=-1.0, scalar2=1.0,
                op0=Alu.mult, op1=Alu.add,
            )
            # w_pre = hp * (1 - E') + rem * at
            notE = small.tile([BL, T], f32)
            nc.vector.tensor_scalar(
                out=notE[:], in0=E[:], scalar1=-1.0, scalar2=1.0,
                op0=Alu.mult, op1=Alu.add,
            )
            w1 = small.tile([BL, T], f32)
            nc.vector.tensor_mul(w1[:], hp[:], notE[:])
            w2 = small.tile([BL, T], f32)
            nc.vector.tensor_mul(w2[:], rem[:], at[:])
            wp = small.tile([BL, T], f32)
            nc.vector.tensor_add(wp[:], w1[:], w2[:])
            nc.vector.tensor_mul(wp[:], wp[:], sw[:])
            # normalize: w / max(sum(w), EPS)
            sums = small.tile([BL, 1], f32)
            nc.vector.reduce_sum(sums[:], wp[:], axis=mybir.AxisListType.X)
            nc.vector.tensor_scalar_max(sums[:], sums[:], EPSILON)
            inv = small.tile([BL, 1], f32)
            nc.vector.reciprocal(inv[:], sums[:])
            wgt = small.tile([BL, T], f32)
            nc.vector.tensor_scalar_mul(wgt[:], wp[:], inv[:])

            # wstack[64g + t, m] = weights[2m + g, t], via PE transpose +
            # two DVE copies (one partition-shifted) -- no DRAM roundtrip.
            wgtT_ps = psump.tile([T, BL], f32)
            nc.tensor.transpose(wgtT_ps[:], wgt[:], ident[:])
            wgtT3 = wgtT_ps[:].rearrange("t (m g) -> t g m", g=2)
            wstack = small.tile([P, PAIRS], f32)
            nc.vector.tensor_copy(wstack[0:T, :], wgtT3[:, 0, :])
            nc.vector.tensor_copy(wstack[T:P, :], wgtT3[:, 1, :])
            # Non-critical small outputs.
            nc.scalar.dma_start(w_d[:], wgt[:])
            # ponder = sum(weights * (1..T))
            pond_t = small.tile([BL, T], f32)
            nc.vector.tensor_mul(pond_t[:], wgt[:], steps_sb[:])
            pond = small.tile([BL, 1], f32)
            nc.vector.reduce_sum(pond[:], pond_t[:], axis=mybir.AxisListType.X)
            nc.scalar.dma_start(pond_d[:], pond[:])
            # Block-diagonal lhsT for all pairs: bd_all = masks * wstack[:, m]
            # (written as float32r so the PE can stream it single-pass)
            f32r = mybir.dt.float32r
            bd_all = small.tile([P, PAIRS * BL], f32r)
            nc.vector.tensor_tensor(
                bd_all[:].rearrange("p (m c) -> p m c", c=BL),
                masks_sb[:].rearrange("p (m c) -> p m c", c=BL),
                wstack[:].unsqueeze(2).broadcast_to([P, PAIRS, BL]),
                Alu.mult,
            )

            # PE warm-up: dense burst of junk matmuls gated on E (ready just
            # before bd_all) so the HAM clock is at 2.4 GHz when the real
            # accumulation starts, and stream pacing keeps it warm after.
            warm_ps = psump.tile([T, 256], f32)
            for _ in range(16):
                nc.tensor.matmul(
                    warm_ps[:], E[:, 0:T], masks_sb[0:BL, 0:256],
                    start=True, stop=True,
                )

            # ---- Phase B: final[b, d] = sum_t wgt[b, t] * outputs[b, t, d]
            outs_flat = out_d[:].rearrange("b t d -> (b t) d")  # [BL*T, D]
            psum_fin = psump.tile([BL, D], f32)
            for m in range(PAIRS):
                rhs = rhsp.tile([P, D], f32r)
                nc.sync.dma_start(
                    rhs[:], outs_flat[m * P : (m + 1) * P, :].bitcast(f32r)
                )
                for j in range(D // NCHUNK):
                    # float32r: fp32 data, single-pass PE streaming (4x fp32)
                    nc.tensor.matmul(
                        psum_fin[:, j * NCHUNK : (j + 1) * NCHUNK],
                        bd_all[:, m * BL : (m + 1) * BL],
                        rhs[:, j * NCHUNK : (j + 1) * NCHUNK],
                        start=(m == 0),
                        stop=(m == PAIRS - 1),
                    )
            # Per-bank copy + store: bank j drains as soon as its (m=15, j)
            # matmul retires, overlapping the remaining banks' matmuls.
            fin_sb = foutp.tile([BL, D], f32)
            for j in range(D // NCHUNK):
                sl = slice(j * NCHUNK, (j + 1) * NCHUNK)
                nc.vector.tensor_copy(fin_sb[:, sl], psum_fin[:, sl])
                nc.scalar.dma_start(fin_d[:, sl], fin_sb[:, sl])

    nc.finalize()
    return nc


def kernel(halt_probs, outputs, step_weights):
    from concourse.bass_utils import run_bass_kernel_spmd

    halt_probs = np.ascontiguousarray(np.asarray(halt_probs, dtype=np.float32))
    outputs = np.ascontiguousarray(np.asarray(outputs, dtype=np.float32))
    step_weights = np.ascontiguousarray(np.asarray(step_weights, dtype=np.float32))

    if "nc" not in _CACHE:
        _CACHE["nc"] = _build()
    nc = _CACHE["nc"]

    core_ids = list(range(NCORES))
    in_maps = []
    for i in core_ids:
        s = slice(i * BL, (i + 1) * BL)
        in_maps.append(
            {
                "halt_probs": np.ascontiguousarray(halt_probs[s]),
                "outputs": np.ascontiguousarray(outputs[s]),
                "step_weights": np.ascontiguousarray(step_weights[s]),
            }
        )
    res = run_bass_kernel_spmd(nc, in_maps, core_ids)
    final = np.concatenate([res.results[i]["final"] for i in core_ids], axis=0)
    ponder = np.concatenate(
        [res.results[i]["ponder"][:, 0] for i in core_ids], axis=0
    )
    weights = np.concatenate([res.results[i]["weights"] for i in core_ids], axis=0)
    return final, ponder, weights


# revision 15
# speedup vs baseline: 1.5337x; 1.0476x over previous
"""ACT halting-weights kernel for 8 TRN2 NeuronCores (pure data parallel over B)."""

import sys

for _p in ("/opt/trn_rl_repo", "/root/.axon_site"):
    if _p not in sys.path:
        sys.path.insert(0, _p)

import numpy as np

B, T, D = 256, 64, 2048
NCORES = 8
BL = B // NCORES          # 32 rows per core
P = 128                   # SBUF partitions
PAIRS = BL // 2           # 16 row-pairs per core; each pair = 128 partitions of (b, t)
NCHUNK = 512              # fp32 PSUM bank width
THRESHOLD = 0.99
EPSILON = 0.01

_CACHE = {}


def _build():
    import concourse.tile as tile
    from concourse import bacc, mybir

    f32 = mybir.dt.float32
    Alu = mybir.AluOpType

    nc = bacc.Bacc()
    hp_d = nc.dram_tensor("halt_probs", [BL, T, 1], f32, kind="ExternalInput")
    out_d = nc.dram_tensor("outputs", [BL, T, D], f32, kind="ExternalInput")
    sw_d = nc.dram_tensor("step_weights", [BL, T], f32, kind="ExternalInput")
    fin_d = nc.dram_tensor("final", [BL, D], f32, kind="ExternalOutput")
    pond_d = nc.dram_tensor("ponder", [BL, 1], f32, kind="ExternalOutput")
    w_d = nc.dram_tensor("weights", [BL, T], f32, kind="ExternalOutput")

    # Block-diagonal placement masks: for pair m, column 2m+g (within that
    # pair's [P, BL] lhsT slice) is 1 on partitions [64g, 64g+64).
    masks_np = np.zeros((P, PAIRS * BL), np.float32)
    for m in range(PAIRS):
        masks_np[0:64, m * BL + 2 * m] = 1.0
        masks_np[64:128, m * BL + 2 * m + 1] = 1.0
    masks_d = nc.inline_tensor(masks_np, name="masks")
    steps_np = np.broadcast_to(
        np.arange(1, T + 1, dtype=np.float32), (BL, T)
    ).copy()
    steps_d = nc.inline_tensor(steps_np, name="steps")
    ident_d = nc.inline_tensor(np.eye(BL, dtype=np.float32), name="ident")

    with tile.TileContext(nc) as tc:
        with (
            tc.tile_pool(name="small", bufs=1) as small,
            tc.tile_pool(name="rhs", bufs=10) as rhsp,
            tc.tile_pool(name="psum", bufs=1, space="PSUM") as psump,
            tc.tile_pool(name="fout", bufs=1) as foutp,
        ):
            # ---- Phase A: per-row halting weights ([BL, T], b on partitions)
            # Small DMAs ride the ACT HWDGE ring so they never queue behind
            # the 1 MB outputs stream on the SP ring.
            hp = small.tile([BL, T], f32)
            nc.scalar.dma_start(hp[:], hp_d[:].rearrange("b t one -> b (t one)"))
            sw = small.tile([BL, T], f32)
            nc.scalar.dma_start(sw[:], sw_d[:])
            steps_sb = small.tile([BL, T], f32)
            nc.scalar.dma_start(steps_sb[:], steps_d[:])
            masks_sb = small.tile([P, PAIRS * BL], f32)
            nc.scalar.dma_start(masks_sb[:], masks_d[:])

            ident = small.tile([BL, BL], f32)
            nc.scalar.dma_start(ident[:], ident_d[:])

            cum = small.tile([BL, T], f32)
            nc.vector.tensor_tensor_scan(
                cum[:], hp[:], hp[:], 0.0, Alu.add, Alu.bypass
            )
            # E' = (cum >= THRESHOLD) with forced last step (halting mask)
            E = small.tile([BL, T], f32)
            nc.vector.tensor_scalar(
                out=E[:], in0=cum[:], scalar1=THRESHOLD, scalar2=None, op0=Alu.is_ge
            )
            nc.vector.memset(E[:, T - 1 : T], 1.0)
            # cumprev = cum - hp (= cumsum up to t-1); rem = 1 - cumprev
            cumprev = small.tile([BL, T], f32)
            nc.vector.tensor_sub(cumprev[:], cum[:], hp[:])
            # at = E' * (1 - Eprev): first step where E' holds, elementwise
            notEp = small.tile([BL, T], f32)
            nc.vector.tensor_scalar(
                out=notEp[:], in0=cumprev[:], scalar1=THRESHOLD, scalar2=None,
                op0=Alu.is_lt,
            )
            at = small.tile([BL, T], f32)
            nc.vector.tensor_mul(at[:], E[:], notEp[:])
            rem = small.tile([BL, T], f32)
            nc.vector.tensor_scalar(
                out=rem[:], in0=cumprev[:], scalar1=-1.0, scalar2=1.0,
                op0=Alu.mult, op1=Alu.add,
            )
            # w_pre = hp * (1 - E') + rem * at
            notE = small.tile([BL, T], f32)
            nc.vector.tensor_scalar(
                out=notE[:], in0=E[:], scalar1=-1.0, scalar2=1.0,
                op0=Alu.mult, op1=Alu.add,
            )
            w1 = small.tile([BL, T], f32)
            nc.vector.tensor_mul(w1[:], hp[:], notE[:])
            w2 = small.tile([BL, T], f32)
            nc.vector.tensor_mul(w2[:], rem[:], at[:])
            wp = small.tile([BL, T], f32)
            nc.vector.tensor_add(wp[:], w1[:], w2[:])
            nc.vector.tensor_mul(wp[:], wp[:], sw[:])
            # normalize: w / max(sum(w), EPS)
            sums = small.tile([BL, 1], f32)
            nc.vector.reduce_sum(sums[:], wp[:], axis=mybir.AxisListType.X)
            nc.vector.tensor_scalar_max(sums[:], sums[:], EPSILON)
            inv = small.tile([BL, 1], f32)
            nc.vector.reciprocal(inv[:], sums[:])
            wgt = small.tile([BL, T], f32)
            nc.vector.tensor_scalar_mul(wgt[:], wp[:], inv[:])

            # wstack[64g + t, m] = weights[2m + g, t], via PE transpose +
            # two DVE copies (one partition-shifted) -- no DRAM roundtrip.
            wgtT_ps = psump.tile([T, BL], f32)
            nc.tensor.transpose(wgtT_ps[:], wgt[:], ident[:])
            wgtT3 = wgtT_ps[:].rearrange("t (m g) -> t g m", g=2)
            wstack = small.tile([P, PAIRS], f32)
            nc.vector.tensor_copy(wstack[0:T, :], wgtT3[:, 0, :])
            nc.vector.tensor_copy(wstack[T:P, :], wgtT3[:, 1, :])
            # Non-critical small outputs.
            nc.scalar.dma_start(w_d[:], wgt[:])
            # ponder = sum(weights * (1..T))
            pond_t = small.tile([BL, T], f32)
            nc.vector.tensor_mul(pond_t[:], wgt[:], steps_sb[:])
            pond = small.tile([BL, 1], f32)
            nc.vector.reduce_sum(pond[:], pond_t[:], axis=mybir.AxisListType.X)
            nc.scalar.dma_start(pond_d[:], pond[:])
            # Block-diagonal lhsT for all pairs: bd_all = masks * wstack[:, m]
            # (written as float32r so the PE can stream it single-pass)
            f32r = mybir.dt.float32r
            bd_all = small.tile([P, PAIRS * BL], f32r)
            nc.vector.tensor_tensor(
                bd_all[:].rearrange("p (m c) -> p m c", c=BL),
                masks_sb[:].rearrange("p (m c) -> p m c", c=BL),
                wstack[:].unsqueeze(2).broadcast_to([P, PAIRS, BL]),
                Alu.mult,
            )

            # ---- Phase B: final[b, d] = sum_t wgt[b, t] * outputs[b, t, d]
            outs_flat = out_d[:].rearrange("b t d -> (b t) d")  # [BL*T, D]
            NJ = D // NCHUNK
            # One PSUM tile per fp32 bank so bank j's drain only depends on
            # its own last accumulating matmul, not the whole [BL, D] region.
            psum_banks = [
                psump.tile([BL, NCHUNK], f32, name=f"pfin{j}", tag=f"pfin{j}") for j in range(NJ)
            ]
            fin_sb = foutp.tile([BL, D], f32)

            def mm(m, j, rhs_ap):
                nc.tensor.matmul(
                    psum_banks[j][:],
                    bd_all[:, m * BL : (m + 1) * BL],
                    rhs_ap,
                    start=(m == 0),
                    stop=(m == PAIRS - 1),
                )

            for m in range(PAIRS - 1):
                rhs = rhsp.tile([P, D], f32r)
                nc.sync.dma_start(
                    rhs[:], outs_flat[m * P : (m + 1) * P, :].bitcast(f32r)
                )
                for j in range(NJ):
                    mm(m, j, rhs[:, j * NCHUNK : (j + 1) * NCHUNK])
            # Last pair arrives as 4 quarter-tiles: each matmul + drain fires
            # as soon as its slice lands, shrinking the post-stream tail.
            mlast = PAIRS - 1
            for j in range(NJ):
                sl = slice(j * NCHUNK, (j + 1) * NCHUNK)
                rhs_q = rhsp.tile([P, NCHUNK], f32r, name=f"rhsq{j}", tag=f"rhsq{j}")
                nc.sync.dma_start(
                    rhs_q[:], outs_flat[mlast * P : (mlast + 1) * P, sl].bitcast(f32r)
                )
                mm(mlast, j, rhs_q[:])
                nc.vector.tensor_copy(fin_sb[:, sl], psum_banks[j][:])
                nc.scalar.dma_start(fin_d[:, sl], fin_sb[:, sl])

    nc.finalize()
    return nc


def kernel(halt_probs, outputs, step_weights):
    from concourse.bass_utils import run_bass_kernel_spmd

    halt_probs = np.ascontiguousarray(np.asarray(halt_probs, dtype=np.float32))
    outputs = np.ascontiguousarray(np.asarray(outputs, dtype=np.float32))
    step_weights = np.ascontiguousarray(np.asarray(step_weights, dtype=np.float32))

    if "nc" not in _CACHE:
        _CACHE["nc"] = _build()
    nc = _CACHE["nc"]

    core_ids = list(range(NCORES))
    in_maps = []
    for i in core_ids:
        s = slice(i * BL, (i + 1) * BL)
        in_maps.append(
            {
                "halt_probs": np.ascontiguousarray(halt_probs[s]),
                "outputs": np.ascontiguousarray(outputs[s]),
                "step_weights": np.ascontiguousarray(step_weights[s]),
            }
        )
    res = run_bass_kernel_spmd(nc, in_maps, core_ids)
    final = np.concatenate([res.results[i]["final"] for i in core_ids], axis=0)
    ponder = np.concatenate(
        [res.results[i]["ponder"][:, 0] for i in core_ids], axis=0
    )
    weights = np.concatenate([res.results[i]["weights"] for i in core_ids], axis=0)
    return final, ponder, weights


# revision 16
# speedup vs baseline: 1.6099x; 1.0497x over previous
"""ACT halting-weights kernel for 8 TRN2 NeuronCores (pure data parallel over B)."""

import sys

for _p in ("/opt/trn_rl_repo", "/root/.axon_site"):
    if _p not in sys.path:
        sys.path.insert(0, _p)

import numpy as np

B, T, D = 256, 64, 2048
NCORES = 8
BL = B // NCORES          # 32 rows per core
P = 128                   # SBUF partitions
PAIRS = BL // 2           # 16 row-pairs per core; each pair = 128 partitions of (b, t)
NCHUNK = 512              # fp32 PSUM bank width
THRESHOLD = 0.99
EPSILON = 0.01

_CACHE = {}


def _build():
    import concourse.bass as bass_mod
    import concourse.tile as tile
    from concourse import bacc, mybir

    f32 = mybir.dt.float32
    Alu = mybir.AluOpType

    # Skip the ~3.4us construction-time all-engine barrier: it only fences
    # the builtin const-tile memsets, which this kernel never reads.
    _orig_barrier = bass_mod.Bass.all_engine_barrier
    bass_mod.Bass.all_engine_barrier = lambda self, **kw: None
    try:
        nc = bacc.Bacc()
    finally:
        bass_mod.Bass.all_engine_barrier = _orig_barrier
    hp_d = nc.dram_tensor("halt_probs", [BL, T, 1], f32, kind="ExternalInput")
    out_d = nc.dram_tensor("outputs", [BL, T, D], f32, kind="ExternalInput")
    sw_d = nc.dram_tensor("step_weights", [BL, T], f32, kind="ExternalInput")
    fin_d = nc.dram_tensor("final", [BL, D], f32, kind="ExternalOutput")
    pond_d = nc.dram_tensor("ponder", [BL, 1], f32, kind="ExternalOutput")
    w_d = nc.dram_tensor("weights", [BL, T], f32, kind="ExternalOutput")

    # Block-diagonal placement masks: for pair m, column 2m+g (within that
    # pair's [P, BL] lhsT slice) is 1 on partitions [64g, 64g+64).
    masks_np = np.zeros((P, PAIRS * BL), np.float32)
    for m in range(PAIRS):
        masks_np[0:64, m * BL + 2 * m] = 1.0
        masks_np[64:128, m * BL + 2 * m + 1] = 1.0
    masks_d = nc.inline_tensor(masks_np, name="masks")
    steps_np = np.broadcast_to(
        np.arange(1, T + 1, dtype=np.float32), (BL, T)
    ).copy()
    steps_d = nc.inline_tensor(steps_np, name="steps")
    ident_d = nc.inline_tensor(np.eye(BL, dtype=np.float32), name="ident")

    with tile.TileContext(nc) as tc:
        with (
            tc.tile_pool(name="small", bufs=1) as small,
            tc.tile_pool(name="rhs", bufs=10) as rhsp,
            tc.tile_pool(name="psum", bufs=1, space="PSUM") as psump,
            tc.tile_pool(name="fout", bufs=1) as foutp,
        ):
            # ---- Phase A: per-row halting weights ([BL, T], b on partitions)
            # Small DMAs ride the ACT HWDGE ring so they never queue behind
            # the 1 MB outputs stream on the SP ring.
            hp = small.tile([BL, T], f32)
            nc.scalar.dma_start(hp[:], hp_d[:].rearrange("b t one -> b (t one)"))
            sw = small.tile([BL, T], f32)
            nc.scalar.dma_start(sw[:], sw_d[:])
            steps_sb = small.tile([BL, T], f32)
            nc.scalar.dma_start(steps_sb[:], steps_d[:])
            masks_sb = small.tile([P, PAIRS * BL], f32)
            nc.scalar.dma_start(masks_sb[:], masks_d[:])

            ident = small.tile([BL, BL], f32)
            nc.scalar.dma_start(ident[:], ident_d[:])

            cum = small.tile([BL, T], f32)
            nc.vector.tensor_tensor_scan(
                cum[:], hp[:], hp[:], 0.0, Alu.add, Alu.bypass
            )
            # E' = (cum >= THRESHOLD) with forced last step (halting mask)
            E = small.tile([BL, T], f32)
            nc.vector.tensor_scalar(
                out=E[:], in0=cum[:], scalar1=THRESHOLD, scalar2=None, op0=Alu.is_ge
            )
            nc.vector.memset(E[:, T - 1 : T], 1.0)
            # cumprev = cum - hp (= cumsum up to t-1); rem = 1 - cumprev
            cumprev = small.tile([BL, T], f32)
            nc.vector.tensor_sub(cumprev[:], cum[:], hp[:])
            # at = E' * (1 - Eprev): first step where E' holds, elementwise
            notEp = small.tile([BL, T], f32)
            nc.vector.tensor_scalar(
                out=notEp[:], in0=cumprev[:], scalar1=THRESHOLD, scalar2=None,
                op0=Alu.is_lt,
            )
            at = small.tile([BL, T], f32)
            nc.vector.tensor_mul(at[:], E[:], notEp[:])
            rem = small.tile([BL, T], f32)
            nc.vector.tensor_scalar(
                out=rem[:], in0=cumprev[:], scalar1=-1.0, scalar2=1.0,
                op0=Alu.mult, op1=Alu.add,
            )
            # w_pre = hp * (1 - E') + rem * at
            notE = small.tile([BL, T], f32)
            nc.vector.tensor_scalar(
                out=notE[:], in0=E[:], scalar1=-1.0, scalar2=1.0,
                op0=Alu.mult, op1=Alu.add,
            )
            w1 = small.tile([BL, T], f32)
            nc.vector.tensor_mul(w1[:], hp[:], notE[:])
            w2 = small.tile([BL, T], f32)
            nc.vector.tensor_mul(w2[:], rem[:], at[:])
            wp = small.tile([BL, T], f32)
            nc.vector.tensor_add(wp[:], w1[:], w2[:])
            nc.vector.tensor_mul(wp[:], wp[:], sw[:])
            # normalize: w / max(sum(w), EPS)
            sums = small.tile([BL, 1], f32)
            nc.vector.reduce_sum(sums[:], wp[:], axis=mybir.AxisListType.X)
            nc.vector.tensor_scalar_max(sums[:], sums[:], EPSILON)
            inv = small.tile([BL, 1], f32)
            nc.vector.reciprocal(inv[:], sums[:])
            wgt = small.tile([BL, T], f32)
            nc.vector.tensor_scalar_mul(wgt[:], wp[:], inv[:])

            # wstack[64g + t, m] = weights[2m + g, t], via PE transpose +
            # two DVE copies (one partition-shifted) -- no DRAM roundtrip.
            wgtT_ps = psump.tile([T, BL], f32)
            nc.tensor.transpose(wgtT_ps[:], wgt[:], ident[:])
            wgtT3 = wgtT_ps[:].rearrange("t (m g) -> t g m", g=2)
            wstack = small.tile([P, PAIRS], f32)
            nc.vector.tensor_copy(wstack[0:T, :], wgtT3[:, 0, :])
            nc.vector.tensor_copy(wstack[T:P, :], wgtT3[:, 1, :])
            # Non-critical small outputs.
            nc.scalar.dma_start(w_d[:], wgt[:])
            # ponder = sum(weights * (1..T))
            pond_t = small.tile([BL, T], f32)
            nc.vector.tensor_mul(pond_t[:], wgt[:], steps_sb[:])
            pond = small.tile([BL, 1], f32)
            nc.vector.reduce_sum(pond[:], pond_t[:], axis=mybir.AxisListType.X)
            nc.scalar.dma_start(pond_d[:], pond[:])
            # Block-diagonal lhsT for all pairs: bd_all = masks * wstack[:, m]
            # (written as float32r so the PE can stream it single-pass)
            f32r = mybir.dt.float32r
            bd_all = small.tile([P, PAIRS * BL], f32r)
            nc.vector.tensor_tensor(
                bd_all[:].rearrange("p (m c) -> p m c", c=BL),
                masks_sb[:].rearrange("p (m c) -> p m c", c=BL),
                wstack[:].unsqueeze(2).broadcast_to([P, PAIRS, BL]),
                Alu.mult,
            )

            # ---- Phase B: final[b, d] = sum_t wgt[b, t] * outputs[b, t, d]
            outs_flat = out_d[:].rearrange("b t d -> (b t) d")  # [BL*T, D]
            NJ = D // NCHUNK
            # One PSUM tile per fp32 bank so bank j's drain only depends on
            # its own last accumulating matmul, not the whole [BL, D] region.
            psum_banks = [
                psump.tile([BL, NCHUNK], f32, name=f"pfin{j}", tag=f"pfin{j}") for j in range(NJ)
            ]
            fin_sb = foutp.tile([BL, D], f32)

            def mm(m, j, rhs_ap):
                nc.tensor.matmul(
                    psum_banks[j][:],
                    bd_all[:, m * BL : (m + 1) * BL],
                    rhs_ap,
                    start=(m == 0),
                    stop=(m == PAIRS - 1),
                )

            for m in range(PAIRS - 1):
                rhs = rhsp.tile([P, D], f32r)
                nc.sync.dma_start(
                    rhs[:], outs_flat[m * P : (m + 1) * P, :].bitcast(f32r)
                )
                for j in range(NJ):
                    mm(m, j, rhs[:, j * NCHUNK : (j + 1) * NCHUNK])
            # Last pair arrives as 4 quarter-tiles: each matmul + drain fires
            # as soon as its slice lands, shrinking the post-stream tail.
            mlast = PAIRS - 1
            for j in range(NJ):
                sl = slice(j * NCHUNK, (j + 1) * NCHUNK)
                rhs_q = rhsp.tile([P, NCHUNK], f32r, name=f"rhsq{j}", tag=f"rhsq{j}")
                nc.sync.dma_start(
                    rhs_q[:], outs_flat[mlast * P : (mlast + 1) * P, sl].bitcast(f32r)
                )
                mm(mlast, j, rhs_q[:])
                nc.vector.tensor_copy(fin_sb[:, sl], psum_banks[j][:])
                nc.scalar.dma_start(fin_d[:, sl], fin_sb[:, sl])

    nc.finalize()
    return nc


def kernel(halt_probs, outputs, step_weights):
    from concourse.bass_utils import run_bass_kernel_spmd

    halt_probs = np.ascontiguousarray(np.asarray(halt_probs, dtype=np.float32))
    outputs = np.ascontiguousarray(np.asarray(outputs, dtype=np.float32))
    step_weights = np.ascontiguousarray(np.asarray(step_weights, dtype=np.float32))

    if "nc" not in _CACHE:
        _CACHE["nc"] = _build()
    nc = _CACHE["nc"]

    core_ids = list(range(NCORES))
    in_maps = []
    for i in core_ids:
        s = slice(i * BL, (i + 1) * BL)
        in_maps.append(
            {
                "halt_probs": np.ascontiguousarray(halt_probs[s]),
                "outputs": np.ascontiguousarray(outputs[s]),
                "step_weights": np.ascontiguousarray(step_weights[s]),
            }
        )
    res = run_bass_kernel_spmd(nc, in_maps, core_ids)
    final = np.concatenate([res.results[i]["final"] for i in core_ids], axis=0)
    ponder = np.concatenate(
        [res.results[i]["ponder"][:, 0] for i in core_ids], axis=0
    )
    weights = np.concatenate([res.results[i]["weights"] for i in core_ids], axis=0)
    return final, ponder, weights


# revision 24
# speedup vs baseline: 3.1875x; 1.9800x over previous
"""ACT halting-weights kernel for 8 TRN2 NeuronCores (pure data parallel over B).

Key optimization (topk_masking): weights are exactly zero for t > halt_step,
and with uniform halt probs the cumsum crosses THRESHOLD after ~2-3 steps.
The host computes the exact halt steps (bit-identical fp32 cumsum), picks the
smallest T_CAP bucket covering max(halt_step)+slack, and the device kernel
only streams outputs[:, :T_CAP, :] -- typically 8/64 of the tensor. All
device-side math (cumsum, cutoff, weights, reduction, ponder) still runs on
the full-T halt_probs/step_weights, so results are exact for any input
(worst-case bucket 64 streams everything).
"""

import sys

for _p in ("/opt/trn_rl_repo", "/root/.axon_site"):
    if _p not in sys.path:
        sys.path.insert(0, _p)

import numpy as np

B, T, D = 256, 64, 2048
NCORES = 8
BL = B // NCORES          # 32 rows per core
P = 128                   # SBUF partitions
NCHUNK = 512              # fp32 PSUM bank width
THRESHOLD = 0.99
EPSILON = 0.01
BUCKETS = (8, 16, 32, 64)

_CACHE = {}


def _build(t_cap):
    import concourse.bass as bass_mod
    import concourse.tile as tile
    from concourse import bacc, mybir

    f32 = mybir.dt.float32
    f32r = mybir.dt.float32r
    Alu = mybir.AluOpType

    G = P // t_cap            # rows packed per [128, D] rhs tile
    NT = BL // G              # rhs tiles per core
    NJ = D // NCHUNK

    # Skip the ~3.4us construction-time all-engine barrier: it only fences
    # the builtin const-tile memsets, which this kernel never reads.
    _orig_barrier = bass_mod.Bass.all_engine_barrier
    bass_mod.Bass.all_engine_barrier = lambda self, **kw: None
    try:
        nc = bacc.Bacc()
    finally:
        bass_mod.Bass.all_engine_barrier = _orig_barrier

    hp_d = nc.dram_tensor("halt_probs", [BL, T, 1], f32, kind="ExternalInput")
    out_d = nc.dram_tensor("outputs", [BL, t_cap, D], f32, kind="ExternalInput")
    sw_d = nc.dram_tensor("step_weights", [BL, T], f32, kind="ExternalInput")
    fin_d = nc.dram_tensor("final", [BL, D], f32, kind="ExternalOutput")
    pond_d = nc.dram_tensor("ponder", [BL, 1], f32, kind="ExternalOutput")
    w_d = nc.dram_tensor("weights", [BL, T], f32, kind="ExternalOutput")
    wtmp_d = nc.dram_tensor("wtmp", [BL * t_cap], f32, kind="Internal")

    # Block-diagonal placement masks: rhs tile m packs rows b = m*G + g on
    # partitions [g*t_cap, (g+1)*t_cap); its lhsT column for that row is
    # m*BL + b.
    masks_np = np.zeros((P, NT * BL), np.float32)
    for m in range(NT):
        for g in range(G):
            b = m * G + g
            masks_np[g * t_cap : (g + 1) * t_cap, m * BL + b] = 1.0
    masks_d = nc.inline_tensor(masks_np, name="masks")
    steps_np = np.broadcast_to(
        np.arange(1, T + 1, dtype=np.float32), (BL, T)
    ).copy()
    steps_d = nc.inline_tensor(steps_np, name="steps")

    with tile.TileContext(nc) as tc:
        with (
            tc.tile_pool(name="small", bufs=1) as small,
            tc.tile_pool(name="rhs", bufs=min(10, max(2, NT))) as rhsp,
            tc.tile_pool(name="psum", bufs=1, space="PSUM") as psump,
            tc.tile_pool(name="fout", bufs=1) as foutp,
        ):
            # ---- Phase A: per-row halting weights ([BL, T], b on partitions)
            # Small DMAs ride the ACT HWDGE ring so they never queue behind
            # the big outputs stream on the SP ring.
            hp = small.tile([BL, T], f32)
            nc.scalar.dma_start(hp[:], hp_d[:].rearrange("b t one -> b (t one)"))
            sw = small.tile([BL, T], f32)
            nc.scalar.dma_start(sw[:], sw_d[:])
            steps_sb = small.tile([BL, T], f32)
            nc.scalar.dma_start(steps_sb[:], steps_d[:])
            masks_sb = small.tile([P, NT * BL], f32)
            nc.scalar.dma_start(masks_sb[:], masks_d[:])

            cum = small.tile([BL, T], f32)
            nc.vector.tensor_tensor_scan(
                cum[:], hp[:], hp[:], 0.0, Alu.add, Alu.bypass
            )
            # E' = (cum >= THRESHOLD) with forced last step (halting mask)
            E = small.tile([BL, T], f32)
            nc.vector.tensor_scalar(
                out=E[:], in0=cum[:], scalar1=THRESHOLD, scalar2=None, op0=Alu.is_ge
            )
            nc.vector.memset(E[:, T - 1 : T], 1.0)
            # cumprev = cum - hp (cumsum up to t-1); rem = 1 - cumprev
            cumprev = small.tile([BL, T], f32)
            nc.vector.tensor_sub(cumprev[:], cum[:], hp[:])
            # at = E' * (Eprev < thr): the first step where E' holds
            notEp = small.tile([BL, T], f32)
            nc.vector.tensor_scalar(
                out=notEp[:], in0=cumprev[:], scalar1=THRESHOLD, scalar2=None,
                op0=Alu.is_lt,
            )
            at = small.tile([BL, T], f32)
            nc.vector.tensor_mul(at[:], E[:], notEp[:])
            rem = small.tile([BL, T], f32)
            nc.vector.tensor_scalar(
                out=rem[:], in0=cumprev[:], scalar1=-1.0, scalar2=1.0,
                op0=Alu.mult, op1=Alu.add,
            )
            # w_pre = hp * (1 - E') + rem * at
            notE = small.tile([BL, T], f32)
            nc.vector.tensor_scalar(
                out=notE[:], in0=E[:], scalar1=-1.0, scalar2=1.0,
                op0=Alu.mult, op1=Alu.add,
            )
            w1 = small.tile([BL, T], f32)
            nc.vector.tensor_mul(w1[:], hp[:], notE[:])
            w2 = small.tile([BL, T], f32)
            nc.vector.tensor_mul(w2[:], rem[:], at[:])
            wp = small.tile([BL, T], f32)
            nc.vector.tensor_add(wp[:], w1[:], w2[:])
            nc.vector.tensor_mul(wp[:], wp[:], sw[:])
            # normalize: w / max(sum(w), EPS)
            sums = small.tile([BL, 1], f32)
            nc.vector.reduce_sum(sums[:], wp[:], axis=mybir.AxisListType.X)
            nc.vector.tensor_scalar_max(sums[:], sums[:], EPSILON)
            inv = small.tile([BL, 1], f32)
            nc.vector.reciprocal(inv[:], sums[:])
            wgt = small.tile([BL, T], f32)
            nc.vector.tensor_scalar_mul(wgt[:], wp[:], inv[:])

            # wstack[g*t_cap + t, m] = weights[m*G + g, t] for t < t_cap:
            # read back from the row-major weights output (roundtrip through
            # DRAM transposes across partitions; ~32 descriptors).
            nc.scalar.dma_start(
                wtmp_d[:].rearrange("(b t) -> b t", t=t_cap), wgt[:, 0:t_cap]
            )
            wstack = small.tile([P, NT], f32)
            nc.scalar.dma_start(
                wstack[:], wtmp_d[:].rearrange("(m p) -> p m", p=P)
            )
            # Non-critical small outputs.
            nc.scalar.dma_start(w_d[:], wgt[:])
            pond_t = small.tile([BL, T], f32)
            nc.vector.tensor_mul(pond_t[:], wgt[:], steps_sb[:])
            pond = small.tile([BL, 1], f32)
            nc.vector.reduce_sum(pond[:], pond_t[:], axis=mybir.AxisListType.X)
            nc.scalar.dma_start(pond_d[:], pond[:])
            # Block-diagonal lhsT for all tiles: bd_all = masks * wstack[:, m]
            # (written as float32r so the PE streams it single-pass)
            bd_all = small.tile([P, NT * BL], f32r)
            nc.vector.tensor_tensor(
                bd_all[:].rearrange("p (m c) -> p m c", c=BL),
                masks_sb[:].rearrange("p (m c) -> p m c", c=BL),
                wstack[:].unsqueeze(2).broadcast_to([P, NT, BL]),
                Alu.mult,
            )

            # ---- Phase B: final[b, d] = sum_t wgt[b, t] * outputs[b, t, d]
            outs_flat = out_d[:].rearrange("b t d -> (b t) d")  # [BL*t_cap, D]
            # One PSUM tile per fp32 bank so bank j's drain only depends on
            # its own last accumulating matmul, not the whole [BL, D] region.
            psum_banks = [
                psump.tile([BL, NCHUNK], f32, name=f"pfin{j}", tag=f"pfin{j}")
                for j in range(NJ)
            ]
            fin_sb = foutp.tile([BL, D], f32)

            def mm(m, j, rhs_ap):
                nc.tensor.matmul(
                    psum_banks[j][:],
                    bd_all[:, m * BL : (m + 1) * BL],
                    rhs_ap,
                    start=(m == 0),
                    stop=(m == NT - 1),
                )

            for m in range(NT - 1):
                rhs = rhsp.tile([P, D], f32r)
                nc.sync.dma_start(
                    rhs[:], outs_flat[m * P : (m + 1) * P, :].bitcast(f32r)
                )
                for j in range(NJ):
                    mm(m, j, rhs[:, j * NCHUNK : (j + 1) * NCHUNK])
            # Last tile arrives as 4 quarter-tiles: each matmul + drain fires
            # as soon as its slice lands, shrinking the post-stream tail.
            mlast = NT - 1
            for j in range(NJ):
                sl = slice(j * NCHUNK, (j + 1) * NCHUNK)
                rhs_q = rhsp.tile(
                    [P, NCHUNK], f32r, name=f"rhsq{j}", tag=f"rhsq{j}"
                )
                nc.sync.dma_start(
                    rhs_q[:],
                    outs_flat[mlast * P : (mlast + 1) * P, sl].bitcast(f32r),
                )
                mm(mlast, j, rhs_q[:])
                nc.vector.tensor_copy(fin_sb[:, sl], psum_banks[j][:])
                nc.scalar.dma_start(fin_d[:, sl], fin_sb[:, sl])

    nc.finalize()
    return nc


def _halt_steps(halt_probs):
    hp = halt_probs[..., 0].astype(np.float32)
    cum = np.cumsum(hp, axis=1, dtype=np.float32)
    ex = cum >= THRESHOLD
    return np.where(ex.any(axis=1), ex.argmax(axis=1), T - 1)


def kernel(halt_probs, outputs, step_weights):
    from concourse.bass_utils import run_bass_kernel_spmd

    halt_probs = np.ascontiguousarray(np.asarray(halt_probs, dtype=np.float32))
    outputs = np.ascontiguousarray(np.asarray(outputs, dtype=np.float32))
    step_weights = np.ascontiguousarray(np.asarray(step_weights, dtype=np.float32))

    # Rows with t > halt_step have exactly zero weight; stream only a
    # bucket-sized prefix that provably covers every row's halt step.
    t_need = int(_halt_steps(halt_probs).max()) + 2
    t_cap = next(bkt for bkt in BUCKETS if bkt >= min(t_need, T))

    if t_cap not in _CACHE:
        _CACHE[t_cap] = _build(t_cap)
    nc = _CACHE[t_cap]

    core_ids = list(range(NCORES))
    in_maps = []
    for i in core_ids:
        s = slice(i * BL, (i + 1) * BL)
        in_maps.append(
            {
                "halt_probs": np.ascontiguousarray(halt_probs[s]),
                "outputs": np.ascontiguousarray(outputs[s, :t_cap]),
                "step_weights": np.ascontiguousarray(step_weights[s]),
            }
        )
    res = None
    for attempt in range(3):
        try:
            res = run_bass_kernel_spmd(nc, in_maps, core_ids)
            break
        except Exception:
            # Sporadic NRT_EXEC_UNIT_UNRECOVERABLE: the NeuronCore needs
            # ~60s to recover; retry rather than failing the call.
            if attempt == 2:
                raise
            import time

            time.sleep(75)
    final = np.concatenate([res.results[i]["final"] for i in core_ids], axis=0)
    ponder = np.concatenate(
        [res.results[i]["ponder"][:, 0] for i in core_ids], axis=0
    )
    weights = np.concatenate([res.results[i]["weights"] for i in core_ids], axis=0)
    return final, ponder, weights


# revision 25
# speedup vs baseline: 3.6377x; 1.1412x over previous
"""ACT halting-weights kernel for 8 TRN2 NeuronCores (pure data parallel over B).

Key optimization (topk_masking): weights are exactly zero for t > halt_step,
and with uniform halt probs the cumsum crosses THRESHOLD after ~2-3 steps.
The host computes the exact halt steps (bit-identical fp32 cumsum), picks the
smallest T_CAP bucket covering max(halt_step)+slack, and the device kernel
only streams outputs[:, :T_CAP, :] -- typically 8/64 of the tensor. All
device-side math (cumsum, cutoff, weights, reduction, ponder) still runs on
the full-T halt_probs/step_weights, so results are exact for any input
(worst-case bucket 64 streams everything).
"""

import sys

for _p in ("/opt/trn_rl_repo", "/root/.axon_site"):
    if _p not in sys.path:
        sys.path.insert(0, _p)

import numpy as np

B, T, D = 256, 64, 2048
NCORES = 8
BL = B // NCORES          # 32 rows per core
P = 128                   # SBUF partitions
NCHUNK = 512              # fp32 PSUM bank width
THRESHOLD = 0.99
EPSILON = 0.01
BUCKETS = (8, 16, 32, 64)

_CACHE = {}


def _build(t_cap):
    import concourse.bass as bass_mod
    import concourse.tile as tile
    from concourse import bacc, mybir

    f32 = mybir.dt.float32
    f32r = mybir.dt.float32r
    Alu = mybir.AluOpType

    G = P // t_cap            # rows packed per [128, D] rhs tile
    NT = BL // G              # rhs tiles per core
    NJ = D // NCHUNK

    # Skip the ~3.4us construction-time all-engine barrier: it only fences
    # the builtin const-tile memsets, which this kernel never reads.
    _orig_barrier = bass_mod.Bass.all_engine_barrier
    bass_mod.Bass.all_engine_barrier = lambda self, **kw: None
    try:
        nc = bacc.Bacc()
    finally:
        bass_mod.Bass.all_engine_barrier = _orig_barrier

    hp_d = nc.dram_tensor("halt_probs", [BL, T, 1], f32, kind="ExternalInput")
    out_d = nc.dram_tensor("outputs", [BL, t_cap, D], f32, kind="ExternalInput")
    sw_d = nc.dram_tensor("step_weights", [BL, T], f32, kind="ExternalInput")
    fin_d = nc.dram_tensor("final", [BL, D], f32, kind="ExternalOutput")
    pond_d = nc.dram_tensor("ponder", [BL, 1], f32, kind="ExternalOutput")
    w_d = nc.dram_tensor("weights", [BL, T], f32, kind="ExternalOutput")
    wtmp_d = nc.dram_tensor("wtmp", [BL * t_cap], f32, kind="Internal")

    # Block-diagonal placement masks: rhs tile m packs rows b = m*G + g on
    # partitions [g*t_cap, (g+1)*t_cap); its lhsT column for that row is
    # m*BL + b.
    masks_np = np.zeros((P, NT * BL), np.float32)
    for m in range(NT):
        for g in range(G):
            b = m * G + g
            masks_np[g * t_cap : (g + 1) * t_cap, m * BL + b] = 1.0
    masks_d = nc.inline_tensor(masks_np, name="masks")
    steps_np = np.broadcast_to(
        np.arange(1, T + 1, dtype=np.float32), (BL, T)
    ).copy()
    steps_d = nc.inline_tensor(steps_np, name="steps")

    with tile.TileContext(nc) as tc:
        with (
            tc.tile_pool(name="small", bufs=1) as small,
            tc.tile_pool(name="rhs", bufs=min(10, max(2, NT))) as rhsp,
            tc.tile_pool(name="psum", bufs=1, space="PSUM") as psump,
            tc.tile_pool(name="fout", bufs=1) as foutp,
        ):
            # ---- Phase A: per-row halting weights ([BL, T], b on partitions)
            # Small DMAs ride the ACT HWDGE ring so they never queue behind
            # the big outputs stream on the SP ring.
            hp = small.tile([BL, T], f32)
            nc.scalar.dma_start(hp[:], hp_d[:].rearrange("b t one -> b (t one)"))
            sw = small.tile([BL, T], f32)
            nc.scalar.dma_start(sw[:], sw_d[:])
            steps_sb = small.tile([BL, T], f32)
            nc.scalar.dma_start(steps_sb[:], steps_d[:])
            masks_sb = small.tile([P, NT * BL], f32)
            nc.scalar.dma_start(masks_sb[:], masks_d[:])

            cum = small.tile([BL, T], f32)
            nc.vector.tensor_tensor_scan(
                cum[:], hp[:], hp[:], 0.0, Alu.add, Alu.bypass
            )
            # E' = (cum >= THRESHOLD) with forced last step (halting mask)
            E = small.tile([BL, T], f32)
            nc.vector.tensor_scalar(
                out=E[:], in0=cum[:], scalar1=THRESHOLD, scalar2=None, op0=Alu.is_ge
            )
            nc.vector.memset(E[:, T - 1 : T], 1.0)
            # cumprev = cum - hp (cumsum up to t-1); rem = 1 - cumprev
            cumprev = small.tile([BL, T], f32)
            nc.vector.tensor_sub(cumprev[:], cum[:], hp[:])
            # at = E' * (Eprev < thr): the first step where E' holds
            notEp = small.tile([BL, T], f32)
            nc.vector.tensor_scalar(
                out=notEp[:], in0=cumprev[:], scalar1=THRESHOLD, scalar2=None,
                op0=Alu.is_lt,
            )
            at = small.tile([BL, T], f32)
            nc.vector.tensor_mul(at[:], E[:], notEp[:])
            rem = small.tile([BL, T], f32)
            nc.vector.tensor_scalar(
                out=rem[:], in0=cumprev[:], scalar1=-1.0, scalar2=1.0,
                op0=Alu.mult, op1=Alu.add,
            )
            # w_pre = hp * (1 - E') + rem * at
            notE = small.tile([BL, T], f32)
            nc.vector.tensor_scalar(
                out=notE[:], in0=E[:], scalar1=-1.0, scalar2=1.0,
                op0=Alu.mult, op1=Alu.add,
            )
            w1 = small.tile([BL, T], f32)
            nc.vector.tensor_mul(w1[:], hp[:], notE[:])
            w2 = small.tile([BL, T], f32)
            nc.vector.tensor_mul(w2[:], rem[:], at[:])
            wp = small.tile([BL, T], f32)
            nc.vector.tensor_add(wp[:], w1[:], w2[:])
            nc.vector.tensor_mul(wp[:], wp[:], sw[:])
            # normalize: w / max(sum(w), EPS)
            sums = small.tile([BL, 1], f32)
            nc.vector.reduce_sum(sums[:], wp[:], axis=mybir.AxisListType.X)
            nc.vector.tensor_scalar_max(sums[:], sums[:], EPSILON)
            inv = small.tile([BL, 1], f32)
            nc.vector.reciprocal(inv[:], sums[:])
            wgt = small.tile([BL, T], f32)
            nc.vector.tensor_scalar_mul(wgt[:], wp[:], inv[:])

            # wstack[g*t_cap + t, m] = weights[m*G + g, t] for t < t_cap.
            # Column m is just rows [m*G, (m+1)*G) x cols [0, t_cap) of wgt in
            # row-major order; SBUF->SBUF DMA does the cross-partition fold
            # without an HBM roundtrip.
            wstack = small.tile([P, NT], f32)
            if NT <= 8:
                for m in range(NT):
                    nc.scalar.dma_start(
                        wstack[:, m : m + 1],
                        wgt[m * G : (m + 1) * G, 0:t_cap],
                    )
            else:
                nc.scalar.dma_start(
                    wtmp_d[:].rearrange("(b t) -> b t", t=t_cap),
                    wgt[:, 0:t_cap],
                )
                nc.scalar.dma_start(
                    wstack[:], wtmp_d[:].rearrange("(m p) -> p m", p=P)
                )
            # Non-critical small outputs.
            nc.scalar.dma_start(w_d[:], wgt[:])
            pond_t = small.tile([BL, T], f32)
            nc.vector.tensor_mul(pond_t[:], wgt[:], steps_sb[:])
            pond = small.tile([BL, 1], f32)
            nc.vector.reduce_sum(pond[:], pond_t[:], axis=mybir.AxisListType.X)
            nc.scalar.dma_start(pond_d[:], pond[:])
            # Block-diagonal lhsT for all tiles: bd_all = masks * wstack[:, m]
            # (written as float32r so the PE streams it single-pass)
            bd_all = small.tile([P, NT * BL], f32r)
            nc.vector.tensor_tensor(
                bd_all[:].rearrange("p (m c) -> p m c", c=BL),
                masks_sb[:].rearrange("p (m c) -> p m c", c=BL),
                wstack[:].unsqueeze(2).broadcast_to([P, NT, BL]),
                Alu.mult,
            )

            # ---- Phase B: final[b, d] = sum_t wgt[b, t] * outputs[b, t, d]
            outs_flat = out_d[:].rearrange("b t d -> (b t) d")  # [BL*t_cap, D]
            # One PSUM tile per fp32 bank so bank j's drain only depends on
            # its own last accumulating matmul, not the whole [BL, D] region.
            psum_banks = [
                psump.tile([BL, NCHUNK], f32, name=f"pfin{j}", tag=f"pfin{j}")
                for j in range(NJ)
            ]
            fin_sb = foutp.tile([BL, D], f32)

            def mm(m, j, rhs_ap):
                nc.tensor.matmul(
                    psum_banks[j][:],
                    bd_all[:, m * BL : (m + 1) * BL],
                    rhs_ap,
                    start=(m == 0),
                    stop=(m == NT - 1),
                )

            for m in range(NT - 1):
                rhs = rhsp.tile([P, D], f32r)
                nc.sync.dma_start(
                    rhs[:], outs_flat[m * P : (m + 1) * P, :].bitcast(f32r)
                )
                for j in range(NJ):
                    mm(m, j, rhs[:, j * NCHUNK : (j + 1) * NCHUNK])
            # Last tile arrives as 4 quarter-tiles: each matmul + drain fires
            # as soon as its slice lands, shrinking the post-stream tail.
            mlast = NT - 1
            for j in range(NJ):
                sl = slice(j * NCHUNK, (j + 1) * NCHUNK)
                rhs_q = rhsp.tile(
                    [P, NCHUNK], f32r, name=f"rhsq{j}", tag=f"rhsq{j}"
                )
                nc.sync.dma_start(
                    rhs_q[:],
                    outs_flat[mlast * P : (mlast + 1) * P, sl].bitcast(f32r),
                )
                mm(mlast, j, rhs_q[:])
                nc.vector.tensor_copy(fin_sb[:, sl], psum_banks[j][:])
                nc.scalar.dma_start(fin_d[:, sl], fin_sb[:, sl])

    nc.finalize()
    return nc


def _halt_steps(halt_probs):
    hp = halt_probs[..., 0].astype(np.float32)
    cum = np.cumsum(hp, axis=1, dtype=np.float32)
    ex = cum >= THRESHOLD
    return np.where(ex.any(axis=1), ex.argmax(axis=1), T - 1)


def kernel(halt_probs, outputs, step_weights):
    from concourse.bass_utils import run_bass_kernel_spmd

    halt_probs = np.ascontiguousarray(np.asarray(halt_probs, dtype=np.float32))
    outputs = np.ascontiguousarray(np.asarray(outputs, dtype=np.float32))
    step_weights = np.ascontiguousarray(np.asarray(step_weights, dtype=np.float32))

    # Rows with t > halt_step have exactly zero weight; stream only a
    # bucket-sized prefix that provably covers every row's halt step.
    t_need = int(_halt_steps(halt_probs).max()) + 2
    t_cap = next(bkt for bkt in BUCKETS if bkt >= min(t_need, T))

    if t_cap not in _CACHE:
        _CACHE[t_cap] = _build(t_cap)
    nc = _CACHE[t_cap]

    core_ids = list(range(NCORES))
    in_maps = []
    for i in core_ids:
        s = slice(i * BL, (i + 1) * BL)
        in_maps.append(
            {
                "halt_probs": np.ascontiguousarray(halt_probs[s]),
                "outputs": np.ascontiguousarray(outputs[s, :t_cap]),
                "step_weights": np.ascontiguousarray(step_weights[s]),
            }
        )
    res = None
    for attempt in range(3):
        try:
            res = run_bass_kernel_spmd(nc, in_maps, core_ids)
            break
        except Exception:
            # Sporadic NRT_EXEC_UNIT_UNRECOVERABLE: the NeuronCore needs
            # ~60s to recover; retry rather than failing the call.
            if attempt == 2:
                raise
            import time

            time.sleep(75)
    final = np.concatenate([res.results[i]["final"] for i in core_ids], axis=0)
    ponder = np.concatenate(
        [res.results[i]["ponder"][:, 0] for i in core_ids], axis=0
    )
    weights = np.concatenate([res.results[i]["weights"] for i in core_ids], axis=0)
    return final, ponder, weights
